# revision 34
# baseline (speedup 1.0000x reference)
"""Multi-head causal attention (B=4,S=1024,D=768,H=12,Dh=64) on 8 trn2 cores.

Sharding: core c handles batch b=c//2 and the 6 heads hs=(c%2)*6 .. hs+6
(head-axis tensor parallel x batch parallel; 8 cores = 4 batches x 2 head-halves).

Per-core on-chip dataflow (bf16 matmul operands, fp32 PSUM accumulation):
  xT [128,6,1024] (host-prepacked bf16), W{q,k,v} host-prepacked [128,6,384]
  qT/kT = W-chunk.T(lhsT) @ xT    -> [64,1024] per head (transposed layout)
  v     = xT-chunk.T @ Wv          -> [1024, 6*65] per t-chunk (65th col = ones)
  scoresT[t,s] computed per head-pair in 9 groups of 512 cols/head, each group
  a [128,2,512] PSUM tile (double-buffered so exp pipelines against matmuls);
  exp via ScalarE Exp(scale=1/8) from PSUM into a flat bf16 SBUF buffer; diag
  chunks masked afterwards on VectorE (multiply by the 0/1 triangle)
  ctxT_aug[65, s] = sum_j v_aug_j(lhsT) @ expT_j  (row 64 = softmax denominator)
  y_aug[h, 0:65, s] DMA'd out bf16; host divides by denominators + transposes.
"""

import threading
from contextlib import ExitStack

import ml_dtypes
import numpy as np

import concourse.bass as bass
import concourse.tile as tile
from concourse import bacc, mybir
from concourse.bass_utils import run_bass_kernel_spmd

B, S, D, H, DH = 4, 1024, 768, 12, 64
NCORES = 8
HL = H // 2          # 6 local heads per core
KC = D // 128        # 6 contraction chunks
NPAIR = HL // 2      # head pairs for qk projection
F32 = mybir.dt.float32
BF16 = mybir.dt.bfloat16


def _attn_groups():
    """Chunk table for one head's scoresT, packed into 9 [128,512] groups.

    A chunk (j, c) is the scoresT tile for t-chunk j (rows j*128..j*128+128)
    and s-range [s0, s0+w) inside output half c (s in [512c, 512c+512)).
    Only causal-relevant chunks exist; `diag` chunks get the triangular mask
    applied to their first 128 columns after exp. Each group holds exactly
    512 columns per head (one PSUM bank per head), so the group pool can be
    double-buffered and exp overlaps the next group's matmuls. Groups are
    ordered so half c=1 completes early (ctx c1 can run during later groups)
    and the last groups hold only c=0 chunks (short tail).
    """
    def chunk(j, c, ps_off):
        s0 = max(512 * c, 128 * j)
        w = 512 * (c + 1) - s0
        return dict(j=j, c=c, s0=s0, w=w, diag=(s0 == 128 * j), ps_off=ps_off)

    groups = [
        [chunk(0, 1, 0)],
        [chunk(1, 1, 0)],
        [chunk(2, 1, 0)],
        [chunk(3, 1, 0)],
        [chunk(4, 1, 0)],
        [chunk(5, 1, 0), chunk(7, 1, 384)],
        [chunk(6, 1, 0), chunk(2, 0, 256)],
        [chunk(0, 0, 0)],
        [chunk(1, 0, 0), chunk(3, 0, 384)],
    ]
    for gi, g in enumerate(groups):
        assert sum(ch["w"] for ch in g) == 512
        for ch in g:
            ch["off"] = 512 * gi + ch["ps_off"]
    return groups


def _emit_kernel(ctx: ExitStack, tc: tile.TileContext, xT, wq0, wqr, wk0, wkr,
                 wv, im, y):
    nc = tc.nc
    groups = _attn_groups()

    # ---- pools ----
    const = ctx.enter_context(tc.tile_pool(name="const", bufs=1))
    xtw = ctx.enter_context(tc.tile_pool(name="xtw", bufs=1))
    qk_pool = ctx.enter_context(tc.tile_pool(name="qk", bufs=1))
    # PSUM budget: pj 2 banks + sg 2x2 + cx 2x1 = 8
    pj = ctx.enter_context(tc.tile_pool(name="pj", bufs=1, space="PSUM"))
    sg = ctx.enter_context(tc.tile_pool(name="sg", bufs=2, space="PSUM"))
    cx = ctx.enter_context(tc.tile_pool(name="cx", bufs=2, space="PSUM"))
    ex = ctx.enter_context(tc.tile_pool(name="ex", bufs=2))
    ysb = ctx.enter_context(tc.tile_pool(name="ysb", bufs=6))

    tri01 = const.tile([128, 128], BF16)   # 1 where s >= t else 0

    # xt is split into per-chunk tiles so a read of an early chunk never
    # waits on a later chunk's DMA (deps are tile-granular); likewise the
    # q/k weights are split into the pair-0 slice (gates the lead-in) and
    # the rest (pairs 1-2, consumed later as fillers).
    xts = [xtw.tile([128, S], BF16, name=f"xt{i}") for i in range(KC)]

    def xt(kc):
        return xts[kc]

    w_q0 = xtw.tile([128, KC, 128], BF16)
    w_k0 = xtw.tile([128, KC, 128], BF16)
    w_qr = xtw.tile([128, KC, 2 * 128], BF16)
    w_kr = xtw.tile([128, KC, 2 * 128], BF16)
    w_v = xtw.tile([128, KC, HL * DH], BF16)

    def wq_slice(pp, kc):
        if pp == 0:
            return w_q0[:, kc, :]
        return w_qr[:, kc, (pp - 1) * 128:pp * 128]

    def wk_slice(pp, kc):
        if pp == 0:
            return w_k0[:, kc, :]
        return w_kr[:, kc, (pp - 1) * 128:pp * 128]

    qT = qk_pool.tile([128, NPAIR, S], BF16)  # partitions: (h%2)*64+e, pair, s
    kT = qk_pool.tile([128, NPAIR, S], BF16)
    v_sb = qk_pool.tile([128, 8, HL * (DH + 1)], BF16)  # [t_rel, t_chunk, h*65+x]

    # ---- t=0: all input DMA kicks first (few, large, spread over queues).
    # Each HWDGE queue interleaves its in-flight transfers across the same
    # 16 HW engines, so a tensor's completion fires near the end of ALL
    # transfers sharing its queue — keep the critical path (xt01, wq)
    # alone at the head of separate queues. The host pre-packs every tensor
    # into its exact SBUF layout so each dma_start is one contiguous-per-
    # partition transfer.
    for kc in range(KC - 1):
        nc.sync.dma_start(out=xts[kc], in_=xT[:, kc, :])
    nc.scalar.dma_start(out=w_q0, in_=wq0[:, :, :])
    nc.scalar.dma_start(out=w_k0, in_=wk0[:, :, :])
    nc.gpsimd.dma_start(out=tri01, in_=im[:, :])
    nc.gpsimd.dma_start(out=xts[KC - 1], in_=xT[:, KC - 1, :])
    nc.scalar.dma_start(out=w_qr, in_=wqr[:, :, :])
    nc.scalar.dma_start(out=w_kr, in_=wkr[:, :, :])
    nc.gpsimd.dma_start(out=w_v, in_=wv[:, :, :])

    # ---- PE filler machinery: engines run their streams in order, so the
    # scores groups (paced by the Scalar-engine exp) need independent matmul
    # work interleaved into the PE stream to avoid idle gaps.
    fillers = []  # list of (est_ns, emit_fn, kind)

    def emit_fillers(budget_ns):
        while fillers and budget_ns > 0:
            est, fn, _ = fillers.pop(0)
            fn()
            budget_ns -= est

    def drain_fillers(kinds=None):
        keep = []
        for u in fillers:
            if kinds is None or u[2] in kinds:
                u[1]()
            else:
                keep.append(u)
        fillers[:] = keep

    def proj_qk_units(pp, kind):
        """q/k projection for pair pp as filler units (kc-outer accumulate,
        LDWEIGHTS shared between the two 512-col output banks). The CAST
        order per tensor matches what the first scores groups consume
        (qT upper half / kT lower half first)."""
        units = []
        for wsl, dst, cast_order in ((wq_slice, qT, (1, 0)),
                                     (wk_slice, kT, (0, 1))):
            pss = [pj.tile([128, 512], F32, tag=f"pjq{i}", name=f"ps{pp}{i}")
                   for i in range(2)]

            def unit(kcs, wsl=wsl, pss=pss, pp=pp, dst=dst, co=cast_order):
                def emit():
                    for kc in kcs:
                        for i, ps in enumerate(pss):
                            nc.tensor.matmul(
                                out=ps,
                                lhsT=wsl(pp, kc),
                                rhs=xt(kc)[:, i * 512:(i + 1) * 512],
                                start=(kc == 0), stop=(kc == KC - 1),
                            )
                    if kcs[-1] == KC - 1:
                        for i in co:
                            nc.vector.tensor_copy(
                                out=dst[:, pp, i * 512:(i + 1) * 512],
                                in_=pss[i])
                return emit
            units.append((900, unit([0, 1]), kind))
            units.append((900, unit([2, 3]), kind))
            units.append((900, unit([4, 5]), kind))
        return units

    def proj_v_unit(j):
        def emit():
            psv = pj.tile([128, HL * DH], F32, tag=f"pjq{j % 2}", name=f"psv{j}")
            for kc in range(KC):
                nc.tensor.matmul(
                    out=psv,
                    lhsT=xt(kc)[:, j * 128:(j + 1) * 128],
                    rhs=w_v[:, kc, :],
                    start=(kc == 0), stop=(kc == KC - 1),
                )
            v_dst = v_sb[:, j, :].rearrange("p (h x) -> p h x", h=HL)
            nc.vector.tensor_copy(
                out=v_dst[:, :, 0:DH],
                in_=psv.rearrange("p (h e) -> p h e", h=HL),
            )
            nc.vector.memset(v_dst[:, :, DH:DH + 1], 1.0)
        return (1300, emit, "v")

    chunks = [ch for g in groups for ch in g]
    c1_chunks = sorted((ch for ch in chunks if ch["c"] == 1),
                       key=lambda t: t["j"])
    c0_chunks = sorted((ch for ch in chunks if ch["c"] == 0),
                       key=lambda t: t["j"])

    # per-(pair-index, head, half) state shared across ctx sub-units
    ctx_state = {}

    def ctx_subunit(hp, h, c, sub, exp_pair, last_evac_on_scalar=False):
        """Context sub-unit: accumulate a subset of half-c chunks for head h.

        sub selects chunks: c=1 -> 'a' = j0..4, 'b' = j5..7 (+evacuate);
        c=0 -> 'a' = j in (0,2), 'b' = j in (1,3) (+evacuate).
        Evacuation casts fp32 PSUM -> bf16 half of the per-head y tile; the
        c=0 evacuation (always last) also kicks the per-head output DMA.
        """
        if c == 1:
            cc = [ch for ch in c1_chunks
                  if (ch["j"] <= 4) == (sub == "a")]
        else:
            cc = [ch for ch in c0_chunks
                  if (ch["j"] in (0, 2)) == (sub == "a")]
        first = (sub == "a")
        last = (sub == "b")
        est = sum(ch["w"] for ch in cc) * 5 // 12 + (500 if last else 100)

        def emit():
            key = (hp, h, c)
            if first:
                ctx_state[key] = cx.tile([DH + 1, 512], F32, tag="cx",
                                         name=f"pc{hp}{h}{c}")
            pc = ctx_state[key]
            for idx, ch in enumerate(cc):
                nc.tensor.matmul(
                    out=pc[:, ch["s0"] - 512 * c: ch["s0"] - 512 * c + ch["w"]],
                    lhsT=v_sb[:, ch["j"], :].rearrange(
                        "p (hh x) -> p hh x", hh=HL)[:, h, :],
                    rhs=exp_pair[:, h % 2, ch["off"]:ch["off"] + ch["w"]],
                    start=(first and idx == 0), stop=(last and idx == len(cc) - 1),
                )
            if last:
                ykey = (hp, h)
                if ykey not in ctx_state:
                    ctx_state[ykey] = ysb.tile([DH + 1, S], BF16, tag="ysb",
                                               name=f"y{hp}{h}")
                yt = ctx_state[ykey]
                # final pair's c0 evacuations split across scalar (free after
                # the last exp) and vector so the two heads drain in parallel
                if last_evac_on_scalar and c == 0 and h % 2 == 0:
                    nc.scalar.copy(out=yt[:, c * 512:(c + 1) * 512], in_=pc)
                else:
                    nc.vector.tensor_copy(out=yt[:, c * 512:(c + 1) * 512],
                                          in_=pc)
                # ship each half as soon as it's done, on the sync HWDGE
                # queue (idle once inputs are in); the final pair's c0
                # halves split across scalar/sync so the kicks overlap
                if last_evac_on_scalar and c == 0 and h % 2 == 0:
                    eng = nc.scalar
                else:
                    eng = nc.sync
                eng.dma_start(out=y[h, :, c * 512:(c + 1) * 512],
                              in_=yt[:, c * 512:(c + 1) * 512])
        return (est, emit, "ctx")

    def ctx_units(hp, exp_pair, scalar_evac=False):
        us = []
        for c, sub in ((1, "a"), (1, "b"), (0, "a"), (0, "b")):
            for a in (0, 1):
                us.append(ctx_subunit(hp, 2 * hp + a, c, sub, exp_pair,
                                      last_evac_on_scalar=scalar_evac))
        return us

    def scores_group(hp, gi, exp_pair):
        """One scores group for both heads of pair hp into one [128,2,512]
        PSUM tile (head A bank 0, head B bank 1). A/B matmuls alternate so
        their K=64 row groups (base_partition 0/64) run concurrently. One
        Exp ACT covers both heads via the 3D AP; causal masking of diag
        chunks happens afterwards on the Vector engine."""
        g = groups[gi]
        ps = sg.tile([128, 2, 512], F32, tag="sg", name=f"sg{hp}_{gi}")
        for i, ch in enumerate(g):
            for a in (0, 1):
                nc.tensor.matmul(
                    out=ps[:, a, ch["ps_off"]:ch["ps_off"] + ch["w"]],
                    lhsT=kT[a * 64:a * 64 + 64, hp,
                            ch["j"] * 128:(ch["j"] + 1) * 128],
                    rhs=qT[a * 64:a * 64 + 64, hp,
                           ch["s0"]:ch["s0"] + ch["w"]],
                    start=(i == 0), stop=(i == len(g) - 1),
                )
        nc.scalar.activation(
            out=exp_pair[:, :, gi * 512:(gi + 1) * 512],
            in_=ps,
            func=mybir.ActivationFunctionType.Exp,
            scale=1.0 / np.sqrt(DH),
        )
        for ch in g:
            if ch["diag"]:
                for a in (0, 1):
                    sl = exp_pair[:, a, ch["off"]:ch["off"] + 128]
                    nc.vector.tensor_mul(sl, sl, tri01)

    # ---- schedule ----
    # lead-in: pair-0 projections pipelined per xt chunk in DMA arrival
    # order (no warm-up -- the chunk stream itself opens the HAM gate).
    # k accumulates in an sg PSUM slot so q (pj banks) and k interleave;
    # pair-0 q CASTs go to the still-idle Scalar engine so the four
    # evacuations run two-by-two.
    pss_q0 = [pj.tile([128, 512], F32, tag=f"pjq{i}", name=f"q0_{i}")
              for i in range(2)]
    ps_k0 = sg.tile([128, 2, 512], F32, tag="sg", name="k0")

    kc_order = [0, 1, 2, KC - 1, 3, 4]  # matches DMA arrival (xt5 on SWDGE)
    for idx, kc in enumerate(kc_order):
        for wsl, pss in ((wq_slice, pss_q0),
                         (wk_slice, [ps_k0[:, 0, :], ps_k0[:, 1, :]])):
            for i in range(2):
                nc.tensor.matmul(
                    out=pss[i],
                    lhsT=wsl(0, kc),
                    rhs=xt(kc)[:, i * 512:(i + 1) * 512],
                    start=(idx == 0), stop=(idx == KC - 1),
                )
    nc.vector.tensor_copy(out=kT[:, 0, 0:512], in_=ps_k0[:, 0, :])
    nc.scalar.copy(out=qT[:, 0, 512:1024], in_=pss_q0[1])
    nc.vector.tensor_copy(out=kT[:, 0, 512:1024], in_=ps_k0[:, 1, :])
    nc.scalar.copy(out=qT[:, 0, 0:512], in_=pss_q0[0])

    # fillers for pair 0: pair-1 projections (data already resident) first,
    # v projections (need wv, lands a bit later) interleaved behind
    p1 = proj_qk_units(1, "proj1")
    vu = [proj_v_unit(j) for j in range(8)]
    for i in range(6):
        fillers.append(p1[i])
        fillers.append(vu[i])
    fillers.extend(vu[6:])

    # scores emitted in blocks of two groups: back-to-back matmul runs hide
    # the K=64 drain + LDWEIGHTS boundary cost, while sg double-buffering
    # keeps both groups' exps pipelined on the Scalar engine.
    blocks = [(0, 1), (2, 3), (4, 5), (6, 7), (8,)]
    for hp in range(NPAIR):
        exp_pair = ex.tile([128, 2, 9 * 512], BF16, tag="exp", name=f"exp{hp}")
        lastp = hp == NPAIR - 1
        for blk in blocks:
            for gi in blk:
                scores_group(hp, gi, exp_pair)
            if lastp:
                # feed the endgame: this pair's ctx sub-units become
                # available as their exp prefixes complete
                cu = ctx_units(hp, exp_pair, scalar_evac=True)
                if blk == (4, 5):
                    fillers[0:0] = cu[0:2]       # c1a A,B (needs G0-4)
                elif blk == (6, 7):
                    fillers[0:0] = cu[2:4]       # c1b A,B (needs G5-6)
                    drain_fillers()              # everything before G8
                elif blk == (8,):
                    for est, fn, _ in cu[4:6]:   # c0a A,B (runs during exp G8)
                        fn()
                    for est, fn, _ in cu[6:8]:   # c0b A,B + evac + DMA
                        fn()
                    continue
            emit_fillers(1500)

        if hp == 0:
            # v and pair-1 proj must be done before pair-1 scores / ctx
            drain_fillers(("v", "proj1"))
            fillers.extend(proj_qk_units(2, "proj2"))
            fillers.extend(ctx_units(0, exp_pair))
        elif hp == 1:
            drain_fillers(("proj2", "ctx"))
            fillers.extend(ctx_units(1, exp_pair))


_PROGRAM = None
_PROGRAM_LOCK = threading.Lock()


def _get_program() -> bass.Bass:
    global _PROGRAM
    with _PROGRAM_LOCK:
        if _PROGRAM is None:
            nc = bacc.Bacc(None, target_bir_lowering=False)
            xT = nc.declare_dram_parameter("xT", [128, KC, S], BF16, isOutput=False)
            wq0 = nc.declare_dram_parameter("wq0", [128, KC, 128], BF16, isOutput=False)
            wqr = nc.declare_dram_parameter("wqr", [128, KC, 256], BF16, isOutput=False)
            wk0 = nc.declare_dram_parameter("wk0", [128, KC, 128], BF16, isOutput=False)
            wkr = nc.declare_dram_parameter("wkr", [128, KC, 256], BF16, isOutput=False)
            wv = nc.declare_dram_parameter("wv", [128, KC, HL * DH], BF16, isOutput=False)
            im = nc.declare_dram_parameter("im", [128, 128], BF16, isOutput=False)
            y = nc.declare_dram_parameter("y_aug", [HL, DH + 1, S], BF16, isOutput=True)
            with tile.TileContext(nc) as tc, ExitStack() as ctx:
                _emit_kernel(ctx, tc, xT, wq0, wqr, wk0, wkr, wv, im, y)
            nc.finalize()  # runs Bacc passes (reg alloc, wait splitting)
            _PROGRAM = nc
    return _PROGRAM


def make_in_maps(x, Wq, Wk, Wv):
    """Per-core input dicts: batch b=core//2, heads (core%2)*6..+6.

    Every tensor is pre-packed into its SBUF layout (partition dim first)
    so each on-device dma_start is a single contiguous transfer.
    """
    bf = ml_dtypes.bfloat16
    t = np.arange(128)
    im = (t[None, :] >= t[:, None]).astype(bf)  # 1 where s >= t
    in_maps = []
    for core in range(NCORES):
        b, hs = core // 2, (core % 2) * HL
        xTc = np.asarray(x[b]).T.astype(bf)              # [768, 1024]
        xTc = np.ascontiguousarray(
            xTc.reshape(KC, 128, S).transpose(1, 0, 2))  # [128, 6, 1024]
        maps = {"xT": xTc, "im": im}
        for name, W in (("wq", Wq), ("wk", Wk), ("wv", Wv)):
            # [6,768,64] -> [768, 6*64] (col = h*64+e) -> [128, 6, 384]
            wf = np.asarray(W[hs:hs + HL]).transpose(1, 0, 2).reshape(D, HL * DH)
            wf = np.ascontiguousarray(
                wf.reshape(KC, 128, HL * DH).transpose(1, 0, 2).astype(bf))
            if name == "wv":
                maps[name] = wf
            else:  # split into pair-0 slice and the rest (pairs 1-2)
                maps[name + "0"] = np.ascontiguousarray(wf[:, :, 0:128])
                maps[name + "r"] = np.ascontiguousarray(wf[:, :, 128:384])
        in_maps.append(maps)
    return in_maps


def assemble_output(per_core_results):
    y_full = np.zeros((B, S, H * DH), np.float32)
    for core in range(NCORES):
        ya = per_core_results[core]["y_aug"].astype(np.float32)  # [6, 65, 1024]
        b, hs = core // 2, (core % 2) * HL
        ctxs = ya[:, 0:DH, :] / ya[:, DH:DH + 1, :]              # [6, 64, 1024]
        y_full[b, :, hs * DH:(hs + HL) * DH] = (
            ctxs.transpose(2, 0, 1).reshape(S, HL * DH))
    return y_full


def kernel(x, Wq, Wk, Wv):
    nc = _get_program()
    in_maps = make_in_maps(x, Wq, Wk, Wv)
    res = run_bass_kernel_spmd(nc, in_maps, core_ids=list(range(NCORES)))
    return assemble_output(res.results)
